# revision 19
# baseline (speedup 1.0000x reference)
"""ColorDenseCRFLoss on 8 Trainium2 NeuronCores — dual-engine exp redesign.

Math: loss = -W/N * sum_n sum_ij K_ij S_ij, where for each image n
  K_ij = exp(-0.5*||f_i - f_j||^2)   (f = nearest-downsampled RGB / 15, P=4096 pts)
  S_ij = sum_k seg_k,i seg_k,j       (seg = bilinear-downsampled softmax, K=21)
Bilinear downsample at exactly 2x == 2x2 average pooling; nearest == stride-2.

Sharding: 2 cores per image (batch N=4 -> 8 cores). Symmetry via circulant
blocks: core h of image n owns row-blocks v=0..15 (of its rotated frame,
rotation 2048*h points) with column window d=0..16 (2176 cols); d=0 and d=16
blocks are weighted 1/2 in the reduce and the grand total is doubled.

The PE emits pG = A*g + B' (A=128/ln2, B' = 127*128 - 7; scale and bias are
folded in as two extra contraction rows, K=17), so TWO engines convert PSUM
tiles to K in parallel:
 - ACT: exact exp via activation(Exp, scale=ln2/128, bias=-B'*ln2/128)
 - DVE: Schraudolph bf16: int16(max(pG, 0)) bitcast as bf16 (convert is RNE
   and saturating; the -7 chord-bias correction makes the kernel-weighted
   sum error ~3e-4 even if ALL elements used it).
GPSIMD cannot touch PSUM and its tensor ops are ~15 ns/elem, so it only
issues DMAs. The DVE also runs the multiply-accumulate reduce against segR,
a sliding-union seg layout ([128, 3712]: band q holds seg cols 128q..) that
makes every reduce window a uniform column slice and cuts seg DMA bytes 2.3x
vs materialized per-group windows.

Hardware facts this schedule is built around (measured):
 - Each DMA instruction streams on ONE ring (~22 GB/s) with ~3.5us latency;
   rings of the three DMA-capable queues (SP/ACT/Pool) run in parallel, so
   large tensors are split into column-part DMAs, priority-ordered.
 - PE matmul streams overlap across tile_positions only while their
   stationary weights stay resident; G (rows 32q x all cols) and AS
   (all rows x cols 32q) tiles physically overlap in the array, so the
   schedule keeps G quads/AS quads contiguous per step.
 - Concurrent matmul writers must target distinct PSUM banks unless their
   out partitions are disjoint; G pairs go to separate banks of [128,1024]
   pair tiles, AS band outputs share one bank at disjoint partitions.
 - The PE p-state stays at 1.2 GHz (no 2.4 GHz ramp observed).
"""

import sys

for _p in ("/opt/trn_rl_repo",):
    if _p not in sys.path:
        sys.path.insert(0, _p)

import numpy as np
import ml_dtypes

import bass_rust
import concourse.bass as bass
import concourse.mybir as mybir
from concourse.tile import TileContext
from concourse.bass_utils import run_bass_kernel_spmd

F32 = mybir.dt.float32
BF16 = mybir.dt.bfloat16
I16 = mybir.dt.int16

WEIGHT = 1e-7
SIGMA_RGB = 15.0
N_IMG = 4
P = 4096          # 64*64 points per image
WIN = 17 * 128    # d = 0..16 column window (2176)

LN2 = float(np.log(2.0))
A_SCALE = 128.0 / LN2            # Schraudolph exponent scale
C_ADJ = -7.0                     # chord-bias correction (calibrated)
B_BIAS = 127.0 * 128.0 + C_ADJ   # 16249
S_INV = LN2 / 128.0              # ACT inverse scale
BIAS_INV = -B_BIAS * LN2 / 128.0  # ACT inverse bias

# Per-group convert assignment: (step, tile, lo, hi, engine)
#   step: 0..3 = 512-col chunks, 4 = d16 (512-wide packed tile)
#   tile: 0 = kt1 (row-blocks q0/q1), 1 = kt2 (q2/q3); d16 has a single tile
#   engine: 'A' = ACT exact exp, 'V' = DVE schraudolph, 'G' = gpsimd schraudolph
# Per-group convert assignment: step s (0..4) -> engine.
# Steps 0..3 are 4x512-col G quads in one [128,2048] PSUM tile (one bank per
# concurrent PE stream); step 4 is the 4x128-col d16 boundary.
ASSIGN = [("A", "A"), ("A", "A"), ("A", "A"), ("A", "V"), ("A", "A")]
STEPW = [512, 512, 512, 512, 128]
STEPO = [0, 512, 1024, 1536, 2048]

_CACHED = {}


def _pslice(t, lo, n, c0, c1):
    # [lo:lo+n, c0:c1] partition+col slice; base 96 must be expressed as a
    # double-slice (AP base_partition rejects 96).
    if lo >= 96:
        return t[64:128, c0:c1][lo - 64 : lo - 64 + n, :]
    return t[lo : lo + n, c0:c1]


def _build_nc():
    nc = bass.Bass(trn_type="TRN2", target_bir_lowering=False, debug=False)
    ab_d = nc.dram_tensor("ab", [128, 512], BF16, kind="ExternalInput")
    bb_d = nc.dram_tensor("bb", [4, 17, 4096], BF16, kind="ExternalInput")
    st_d = nc.dram_tensor("segT", [128, 512], BF16, kind="ExternalInput")
    sk_d = nc.dram_tensor("segR", [128, 3712], BF16, kind="ExternalInput")
    out_d = nc.dram_tensor("acc", [128, 32], F32, kind="ExternalOutput")

    EXP = mybir.ActivationFunctionType.Exp
    MULT = mybir.AluOpType.mult
    MAX = mybir.AluOpType.max

    with TileContext(nc) as tc:
        with (
            tc.tile_pool(name="const", bufs=1) as constp,
            tc.tile_pool(name="kt", bufs=8) as ktp,
            tc.tile_pool(name="sc", bufs=2) as scp,
            tc.tile_pool(name="pg", bufs=3, space="PSUM") as pgp,
            tc.tile_pool(name="stk", bufs=2, space="PSUM") as stkp,
        ):
            bias_inv = constp.tile([128, 1], F32, tag="biasi")
            warm = constp.tile([128, 1], F32, tag="warm")
            ab = constp.tile([128, 512], BF16, tag="ab")
            bb = constp.tile([128, 4096], BF16, tag="bb")
            segT = constp.tile([128, 512], BF16, tag="segT")
            segR = constp.tile([128, 3712], BF16, tag="segR")
            accT = constp.tile([128, 32], F32, tag="accT")

            # HBM loads: one DMA instruction streams on ONE ring
            # (~22 GB/s + ~3.5us fixed latency), so every large tensor is
            # split into column-part DMAs that overlap across rings.
            # Priority-ordered round-robin across the three queues; the
            # scalar (ACT) queue gets only the first few so converts start
            # early.
            dmas = []
            dmas.append((ab[:, 0:128], ab_d.ap()[:, 0:128]))
            for q in range(4):   # bb low cols: G groups 0-1
                dmas.append((_pslice(bb, 32 * q, 17, 0, 1024), bb_d.ap()[q][:, 0:1024]))
            dmas.append((segT[:, 0:128], st_d.ap()[:, 0:128]))
            for p in range(4):   # R cols for group 0 reduce: [0:2176]
                dmas.append((segR[:, 544 * p : 544 * p + 544],
                             sk_d.ap()[:, 544 * p : 544 * p + 544]))
            for q in range(4):   # bb mid cols
                dmas.append((_pslice(bb, 32 * q, 17, 1024, 2048), bb_d.ap()[q][:, 1024:2048]))
            dmas.append((ab[:, 128:512], ab_d.ap()[:, 128:512]))
            dmas.append((segT[:, 128:512], st_d.ap()[:, 128:512]))
            dmas.append((segR[:, 2176:2688], sk_d.ap()[:, 2176:2688]))
            for q in range(4):   # bb high cols
                dmas.append((_pslice(bb, 32 * q, 17, 2048, 4096), bb_d.ap()[q][:, 2048:4096]))
            dmas.append((segR[:, 2688:3200], sk_d.ap()[:, 2688:3200]))
            dmas.append((segR[:, 3200:3712], sk_d.ap()[:, 3200:3712]))
            queues = [nc.sync, nc.scalar, nc.gpsimd]
            scalar_budget = 5
            qi = 0
            for dst, srcap in dmas:
                eng = queues[qi % 3]
                if eng is nc.scalar:
                    if scalar_budget == 0:
                        qi += 1
                        eng = queues[qi % 3]
                    else:
                        scalar_budget -= 1
                eng.dma_start(dst, srcap)
                qi += 1
            nc.gpsimd.memset(accT[:], 0.0)
            nc.vector.memset(bias_inv[:], BIAS_INV)
            # one-time ACT exp-table load starts immediately
            nc.scalar.activation(warm[:], bias_inv[:], EXP, bias=bias_inv[:])

            def emit_G(g, s):
                """Row-tiled G matmuls for step s of group g: pairs q0/q1 into
                pg1 bank halves, q2/q3 into pg2; then the convert per ASSIGN."""
                n = STEPW[s]
                pg1 = pgp.tile([128, 1024], F32, tag="pg", name="pg1")
                pg2 = pgp.tile([128, 1024], F32, tag="pg", name="pg2")
                for q in range(4):
                    v = 4 * g + q
                    lo = 128 * v + STEPO[s]
                    pt = pg1 if q < 2 else pg2
                    nc.tensor.matmul(
                        pt[:, 512 * (q % 2) : 512 * (q % 2) + n],
                        _pslice(ab, 32 * q, 17, 128 * g, 128 * g + 128),
                        _pslice(bb, 32 * q, 17, lo, lo + n),
                        start=True,
                        stop=True,
                        tile_position=(32 * q, 0),
                    )
                kt1 = ktp.tile([128, 1024], BF16, tag="kt", name="kt1")
                kt2 = ktp.tile([128, 1024], BF16, tag="kt", name="kt2")
                for (kt, pt), eng in zip(((kt1, pg1), (kt2, pg2)), ASSIGN[s]):
                    if n == 512:
                        src_ap = pt[:]
                        dst_a = kt[:]
                        dst_v = kt[:].bitcast(I16)
                    else:
                        src_ap = pt[:].rearrange("p (a b) -> p a b", b=512)[:, :, 0:n]
                        dst_a = kt[:].rearrange("p (a b) -> p a b", b=512)[:, :, 0:n]
                        dst_v = kt[:].bitcast(I16).rearrange(
                            "p (a b) -> p a b", b=512
                        )[:, :, 0:n]
                    if eng == "A":
                        nc.scalar.activation(
                            dst_a, src_ap, EXP, bias=bias_inv[:], scale=S_INV
                        )
                    else:
                        nc.vector.tensor_scalar(dst_v, src_ap, 0.0, None, MAX)
                return kt1, kt2

            def emit_AS(g, s, kt1, kt2):
                """4-way column-tiled AS matmuls + DVE reduce for step s."""
                n = STEPW[s]
                stk = stkp.tile([128, 512], F32, tag="stk")
                for q in range(4):
                    kt = kt1 if q < 2 else kt2
                    nc.tensor.matmul(
                        _pslice(stk, 32 * q, 32, 0, n),
                        segT[:, 32 * (4 * g + q) : 32 * (4 * g + q) + 32],
                        kt[:, 512 * (q % 2) : 512 * (q % 2) + n],
                        start=True,
                        stop=True,
                        tile_position=(0, 32 * q),
                    )
                sct = scp.tile([128, 512], BF16, tag="sct")
                w0 = 512 * g + STEPO[s]
                col = 8 * g + s

                def stt(lo, hi, scl, acol):
                    nc.vector.scalar_tensor_tensor(
                        out=sct[:, lo:hi],
                        in0=stk[:, lo:hi],
                        scalar=scl,
                        in1=segR[:, w0 + lo : w0 + hi],
                        op0=MULT,
                        op1=MULT,
                        accum_out=accT[:, acol : acol + 1],
                    )

                if s == 0:
                    stt(0, 128, 0.5, 8 * g + 6)
                    stt(128, n, 1.0, col)
                elif s == 4:
                    stt(0, n, 0.5, col)
                else:
                    stt(0, n, 1.0, col)

            prev = None  # (g, s, kt1, kt2)
            for g in range(4):
                for s in range(5):
                    kt1, kt2 = emit_G(g, s)
                    if prev is not None:
                        emit_AS(prev[0], prev[1], prev[2], prev[3])
                    prev = (g, s, kt1, kt2)
            emit_AS(prev[0], prev[1], prev[2], prev[3])

            nc.sync.dma_start(out_d.ap(), accT[:])
    _split_multiwait(nc)
    return nc


def _split_multiwait(nc):
    """walrus encodes at most one semaphore wait per instruction; hoist all
    but one wait onto standalone EventSemaphore instructions placed just
    before the instruction on the same engine queue."""
    ctr = 0
    for f in nc.m.functions:
        for blk in f.blocks:
            insts = blk.instructions
            out = []
            for inst in insts:
                si = inst.sync_info
                if si is not None and len(si.on_wait) > 1:
                    waits = list(si.on_wait)
                    for w in waits[:-1]:
                        es = mybir.InstEventSemaphore(
                            name=f"WSPLIT-{ctr}", ins=[], outs=[]
                        )
                        ctr += 1
                        es.engine = inst.engine
                        es.sync_info = bass_rust.SyncInfo(on_wait=[w], on_update=[])
                        out.append(es)
                    inst.sync_info = bass_rust.SyncInfo(
                        on_wait=[waits[-1]], on_update=list(si.on_update)
                    )
                out.append(inst)
            insts[:] = out


def _host_prep(images, segmentations):
    bf = ml_dtypes.bfloat16
    in_maps = []
    for cidx in range(8):
        n, h = cidx // 2, cidx % 2
        img = images[n][:, ::2, ::2]                       # nearest resize
        img = np.roll(img, -32 * h, axis=1).reshape(3, P)  # circulant rotation
        f = (img / SIGMA_RGB).astype(np.float64)
        f = f - f.mean(axis=1, keepdims=True)              # d2-invariant centering
        sq = (f * f).sum(axis=0)
        ones = np.ones((1, P), np.float64)
        b5 = np.concatenate([f, ones, (-0.5 * sq)[None]], axis=0)
        a5 = np.concatenate([f, (-0.5 * sq)[None], ones], axis=0)

        asc = A_SCALE * a5
        ah = asc.astype(bf)
        al = (asc - ah.astype(np.float64)).astype(bf)
        bh = b5.astype(bf)
        bl = (b5 - bh.astype(np.float64)).astype(bf)

        # ab[q]: rows 0-4 asc_hi, 5-9 asc_lo, 10-14 asc_hi, 15 = 127, 16 = 1
        # cols 128g..128g+127 hold row-block v=4g+q (points 128v..128v+127)
        ab = np.zeros((128, 512), dtype=bf)
        for q in range(4):
            for g in range(4):
                v = 4 * g + q
                cg = slice(128 * g, 128 * g + 128)
                pv = slice(128 * v, 128 * v + 128)
                ab[32 * q + 0 : 32 * q + 5, cg] = ah[:, pv]
                ab[32 * q + 5 : 32 * q + 10, cg] = al[:, pv]
                ab[32 * q + 10 : 32 * q + 15, cg] = ah[:, pv]
            ab[32 * q + 15, :] = bf(127.0)
            ab[32 * q + 16, :] = bf(1.0)
        # bb[q]: rows 0-4 b_hi, 5-9 b_hi, 10-14 b_lo, 15 = 128, 16 = -7
        b17 = np.zeros((17, 4096), dtype=bf)
        b17[0:5] = bh
        b17[5:10] = bh
        b17[10:15] = bl
        b17[15, :] = bf(128.0)
        b17[16, :] = bf(C_ADJ)
        bb = np.ascontiguousarray(np.broadcast_to(b17, (4, 17, 4096)))

        # seg: roll + 2x2 sum pool (/16 folded into final host scale)
        segr = np.roll(segmentations[n], -64 * h, axis=1).astype(np.float64)
        sp = segr.reshape(21, 64, 2, 64, 2).sum(axis=(2, 4)).reshape(21, P)
        spb = sp.astype(bf)

        segT = np.zeros((128, 512), dtype=bf)
        sT = segT.reshape(128, 16, 32)
        for v in range(16):
            sT[:, v, 0:21] = spb[:, 128 * v : 128 * v + 128].T
        segR = np.zeros((128, 3712), dtype=bf)
        for q in range(4):
            w = min(3712, 4096 - 128 * q)
            segR[32 * q : 32 * q + 21, 0:w] = spb[:, 128 * q : 128 * q + w]

        in_maps.append(
            {
                "ab": ab,
                "bb": bb,
                "segT": np.ascontiguousarray(segT),
                "segR": segR,
            }
        )
    return in_maps


def run(images, segmentations, trace=False):
    if "nc" not in _CACHED:
        _CACHED["nc"] = _build_nc()
    nc = _CACHED["nc"]
    in_maps = _host_prep(np.asarray(images), np.asarray(segmentations))
    res = run_bass_kernel_spmd(nc, in_maps, list(range(8)), trace=trace)
    total = np.float64(0.0)
    for r in res.results:
        acc = r["acc"].astype(np.float64)
        total += acc.sum()
    # x2 symmetric halves, /16 unscaled 2x2 pool (quadratic), -W, /N batch mean
    loss = -WEIGHT * 2.0 * total / 16.0 / N_IMG
    return np.array([loss], dtype=np.float32), res


def kernel(images, segmentations):
    out, _ = run(images, segmentations, trace=False)
    return out
